# revision 19
# baseline (speedup 1.0000x reference)
"""Causal self-attention (B=2, T=2048, C=1024, H=16) on 8 trn2 NeuronCores.

Sharding: data-parallel on batch (2 groups of 4 cores) x tensor-parallel on
heads (4 heads per core). Each core computes qkv projection for its heads,
causal attention, and a partial out-projection over its heads' slice of the
hidden dim; the host sums the 4 partials per batch.

Device layout notes:
  - x is pre-transposed on host to xT [C, T] so the C contraction sits on
    SBUF partitions for both qkv matmuls.
  - q and k are produced transposed ([64, T] per head) packed in head-pairs
    into [128, T] tiles; v is produced natural [T, 64] with an appended
    ones column so the softmax denominator falls out of the AV matmul
    (row 64 of the PSUM accumulator).
  - scores are computed transposed [T_k, T_q]; softmax runs without max
    subtraction (logits are O(6) for N(0,1) inputs, safe in fp32 exp).
  - matmuls run in float16 (1 cycle/row on the PE like bf16, but 10
    mantissa bits; all intermediate values here are O(3000) max, far from
    the fp16 range limit). PSUM accumulation is fp32.
"""

import sys

for _p in ("/root/.axon_site", "/root/.axon_site/_ro/trn_rl_repo", "/opt/trn_rl_repo"):
    if _p not in sys.path:
        sys.path.append(_p)

import numpy as np

B, T, C = 2, 2048, 1024
H, D = 16, 64
N_CORES = 8
HEADS_PER_CORE = H // 4  # 4 head-groups x 2 batches = 8 cores

_cache = {}
TRACE = False


def _build(T, C, nhc, D, n_cores):
    """Build + compile the SPMD program. nhc = heads per core (must be even)."""
    import concourse.tile as tile
    from concourse import bacc, mybir

    f32 = mybir.dt.float32
    f16 = mybir.dt.float16
    EXP = mybir.ActivationFunctionType.Exp

    assert nhc % 2 == 0 and D == 64
    npairs = nhc // 2
    F = 2 * nhc * D          # qk projection output rows (q+k for nhc heads)
    VW = nhc * D             # v projection width
    CIN = nhc * D            # local c_in slice for out-proj
    NC_ = C // 128           # contraction tiles
    NF = F // 128            # qk f-tiles (= 2*npairs)
    NTB = T // 512           # 512-wide t-blocks
    NTT = T // 128           # 128-wide t-tiles
    NQ = T // 512            # q-blocks
    CO = min(512, C)         # out-proj column block width
    NCO = C // CO            # out-proj column blocks

    nc = bacc.Bacc("TRN2", target_bir_lowering=False, debug=False,
                   enable_asserts=False, num_devices=n_cores)

    xT = nc.dram_tensor("xT", [C, T], f16, kind="ExternalInput").ap()
    wqk = nc.dram_tensor("wqk", [C, F], f16, kind="ExternalInput").ap()
    wv = nc.dram_tensor("wv", [C, VW], f16, kind="ExternalInput").ap()
    wo = nc.dram_tensor("wo", [CIN, C], f16, kind="ExternalInput").ap()
    tri = nc.dram_tensor("tri", [128, 128], f16, kind="ExternalInput").ap()
    out = nc.dram_tensor("out", [npairs, T, C], f32, kind="ExternalOutput").ap()

    with tile.TileContext(nc) as tc:
        with tc.tile_pool(name="qk", bufs=NF) as qk_pool, \
             tc.tile_pool(name="v", bufs=NTT) as v_pool, \
             tc.tile_pool(name="misc", bufs=1) as misc_pool, \
             tc.tile_pool(name="attn2", bufs=npairs) as attn2_pool, \
             tc.tile_pool(name="wo", bufs=npairs) as wo_pool, \
             tc.tile_pool(name="ob", bufs=3) as ob_pool:

            tri_sb = misc_pool.tile([128, 128], f16, tag="tri")
            nc.sync.dma_start(tri_sb[:], tri[:])

            qk_sb = [qk_pool.tile([128, T], f16, tag="qk", name=f"qk{i}") for i in range(NF)]
            v_sb = [v_pool.tile([128, nhc * 65], f16, tag="v", name=f"v{i}") for i in range(NTT)]
            attn2_sb = [attn2_pool.tile([128, T], f16, tag="attn2",
                                        name=f"attn2_{i}") for i in range(npairs)]
            wo_sb = []
            for pr in range(npairs):
                wt = wo_pool.tile([128, C], f16, tag="wo", name=f"wo{pr}")
                nc.sync.dma_start(wt[:], wo[pr * 128:(pr + 1) * 128, :])
                wo_sb.append(wt)

            QW = min(512, T)
            SJT = QW // 128
            NQB = T // QW
            scale = 1.0 / np.sqrt(D)

            def emit_qk_ftile(f, psp, xT_sb, wqk_sb, nbufs=2):
                """Generator: one qk-proj matmul per next() for f-tile `f`,
                WQ-wide t-blocks (fp16 moving operand allows N=1024)."""
                WQ = min(512, T)
                for tb in range(T // WQ):
                    p = psp.tile([128, WQ], f32, tag="psqk", bufs=nbufs,
                                 name=f"psqk_{f}_{tb}")
                    for c in range(NC_):
                        nc.tensor.matmul(
                            p[:],
                            wqk_sb[c][:, f * 128:(f + 1) * 128],
                            xT_sb[c][:, tb * WQ:(tb + 1) * WQ],
                            start=(c == 0), stop=(c == NC_ - 1))
                        yield
                    nc.vector.tensor_copy(
                        qk_sb[f][:, tb * WQ:(tb + 1) * WQ], p[:])

            def emit_outproj_group(pr, tt, psp):
                """Generator: one out-proj matmul per next() for (pr, tt)."""
                ob = ob_pool.tile([128, C], f32, tag="ob", name=f"ob_{pr}_{tt}")
                WO = min(512, C)
                for co in range(C // WO):
                    p = psp.tile([128, WO], f32, tag="psout", bufs=1,
                                 name=f"pso_{pr}_{tt}_{co}")
                    nc.tensor.matmul(
                        p[:],
                        attn2_sb[pr][:, tt * 128:(tt + 1) * 128],
                        wo_sb[pr][:, co * WO:(co + 1) * WO],
                        start=True, stop=True)
                    nc.vector.tensor_copy(ob[:, co * WO:(co + 1) * WO], p[:])
                    yield
                nc.sync.dma_start(out[pr, tt * 128:(tt + 1) * 128, :], ob[:])

            def chain(gens):
                for g in gens:
                    yield from g

            class FillQueue:
                """FIFO of PE-work generators, pulled between attention
                k-tile steps to keep the PE dense while ACT runs exp."""

                def __init__(self):
                    self.gens = []
                    self.nslot = 0
                    self.npulled = 0

                def add(self, gen):
                    self.gens.append(gen)

                def pull(self, spf):
                    self.nslot += 1
                    while self.gens and self.npulled < self.nslot * spf:
                        try:
                            next(self.gens[0])
                            self.npulled += 1
                        except StopIteration:
                            self.gens.pop(0)

                def drain(self):
                    for g in self.gens:
                        for _ in g:
                            pass
                    self.gens = []

            def attention_pair(pair, psS, psV, r_pool, rb_pool, exp_pool,
                               queue, spf, on_qb_done=None):
                """Attention for both heads of `pair`; q-block outer so
                finished q-blocks feed new filler work via on_qb_done.
                Scores for k-tile jt+1 are emitted before the AV matmul of
                k-tile jt; queue fillers cover the residual ACT latency."""
                Q2 = qk_sb[2 * pair]
                K2 = qk_sb[2 * pair + 1]
                for qb in range(NQB):
                    njt = SJT * (qb + 1)
                    for hh in range(2):
                        h = pair * 2 + hh
                        base = hh * 64
                        qT = Q2[base:base + 64, :]
                        kT = K2[base:base + 64, :]
                        av = psV.tile([65, QW], f32, tag="psav",
                                      name=f"psav_{pair}_{hh}_{qb}")

                        def emit_av(jt, ex, col0, ncols, av=av, h=h, njt=njt):
                            nc.tensor.matmul(
                                av[:, col0:col0 + ncols],
                                v_sb[jt][:, h * 65:(h + 1) * 65],
                                ex[:, col0:col0 + ncols],
                                start=(jt == 0), stop=(jt == njt - 1))

                        pend = None
                        for jt in range(njt):
                            d = jt - SJT * qb
                            col0 = max(d, 0) * 128
                            ncols = QW - col0
                            sc = psS.tile([128, QW], f32, tag="pssc",
                                          name=f"pssc_{pair}_{hh}_{qb}_{jt}")
                            nc.tensor.matmul(
                                sc[:, col0:QW],
                                kT[:, jt * 128:(jt + 1) * 128],
                                qT[:, qb * QW + col0:(qb + 1) * QW],
                                start=True, stop=True)
                            ex = exp_pool.tile([128, QW], f16, tag="ex",
                                               bufs=4,
                                               name=f"ex_{pair}_{hh}_{qb}_{jt}")
                            nc.scalar.activation(
                                ex[:, col0:QW], sc[:, col0:QW], EXP,
                                scale=scale)
                            if d >= 0:
                                nc.vector.tensor_mul(
                                    ex[:, col0:col0 + 128],
                                    ex[:, col0:col0 + 128], tri_sb[:])
                            queue.pull(spf)
                            if pend is not None:
                                emit_av(*pend)
                            pend = (jt, ex, col0, ncols)
                        emit_av(*pend)
                        # stage accumulator to SBUF (frees the PSUM bank),
                        # then normalize rows 0..63 by row 64 (sum of exp)
                        avsb = r_pool.tile([65, QW], f32, tag="avs")
                        nc.vector.tensor_copy(avsb[:], av[:])
                        r1s = r_pool.tile([1, QW], f32, tag="r1s")
                        nc.vector.tensor_copy(r1s[:], avsb[64:65, :])
                        r1 = r_pool.tile([1, QW], f32, tag="r1")
                        nc.vector.reciprocal_approx_fast(r1[:], r1s[:])
                        rb = rb_pool.tile([64, QW], f32, tag="rb")
                        nc.gpsimd.partition_broadcast(rb[:], r1[:])
                        nc.vector.tensor_mul(
                            attn2_sb[pair][base:base + 64,
                                           qb * QW:(qb + 1) * QW],
                            avsb[0:64, :], rb[:])
                    if on_qb_done is not None:
                        on_qb_done(qb)

            # ---- phase 1: loads + qk proj (pair 0) + v proj (all) ----
            with tc.tile_pool(name="xT", bufs=NC_) as xT_pool, \
                 tc.tile_pool(name="wqk", bufs=NC_) as wqk_pool, \
                 tc.tile_pool(name="wv", bufs=NC_) as wv_pool:

                xT_sb, wqk_sb, wv_sb = [], [], []
                for c in range(NC_):
                    w1 = wqk_pool.tile([128, F], f16, tag="wqk")
                    nc.sync.dma_start(w1[:], wqk[c * 128:(c + 1) * 128, :])
                    wqk_sb.append(w1)
                    x1 = xT_pool.tile([128, T], f16, tag="xT")
                    nc.sync.dma_start(x1[:], xT[c * 128:(c + 1) * 128, :])
                    xT_sb.append(x1)
                    w2 = wv_pool.tile([128, VW], f16, tag="wv")
                    nc.sync.dma_start(w2[:], wv[c * 128:(c + 1) * 128, :])
                    wv_sb.append(w2)

                with tc.tile_pool(name="psA", bufs=2, space="PSUM") as psA, \
                     tc.tile_pool(name="psB", bufs=2, space="PSUM") as psB:
                    for f in range(min(2, NF)):
                        for _ in emit_qk_ftile(f, psA, xT_sb, wqk_sb):
                            pass
                    for tt in range(NTT):
                        p = psB.tile([128, VW], f32, tag="psv")
                        for c in range(NC_):
                            nc.tensor.matmul(
                                p[:],
                                xT_sb[c][:, tt * 128:(tt + 1) * 128],
                                wv_sb[c][:],
                                start=(c == 0), stop=(c == NC_ - 1))
                        dst = v_sb[tt][:].rearrange("p (h e) -> p h e", e=65)
                        srcp = p[:].rearrange("p (h e) -> p h e", e=64)
                        nc.vector.tensor_copy(dst[:, :, 0:64], srcp)
                        nc.vector.memset(dst[:, :, 64:65], 1.0)

                # ---- phase 2: attention per pair with PE fillers ----
                with tc.tile_pool(name="exp", bufs=4) as exp_pool, \
                     tc.tile_pool(name="rr", bufs=2) as r_pool, \
                     tc.tile_pool(name="rb", bufs=2) as rb_pool, \
                     tc.tile_pool(name="psS", bufs=4, space="PSUM") as psS, \
                     tc.tile_pool(name="psV", bufs=2, space="PSUM") as psV:

                    for pair in range(npairs):
                        with tc.tile_pool(name=f"psF{pair}", bufs=1,
                                          space="PSUM") as psF:
                            queue = FillQueue()
                            last = pair == npairs - 1
                            if not last:
                                # next pair's qk projection
                                queue.add(emit_qk_ftile(
                                    2 * pair + 2, psF, xT_sb, wqk_sb, 1))
                                queue.add(emit_qk_ftile(
                                    2 * pair + 3, psF, xT_sb, wqk_sb, 1))
                            else:
                                # out-proj of all earlier pairs
                                for pr in range(pair):
                                    for tt in range(NTT):
                                        queue.add(emit_outproj_group(
                                            pr, tt, psF))

                            def on_qb_done(qb, pair=pair, psF=psF,
                                           queue=queue):
                                # the last pair's own out-proj becomes
                                # filler as its q-blocks finish
                                ttp = QW // 128
                                for tt in range(qb * ttp, (qb + 1) * ttp):
                                    queue.add(emit_outproj_group(
                                        pair, tt, psF))

                            attention_pair(pair, psS, psV, r_pool, rb_pool,
                                           exp_pool, queue, 2.0,
                                           on_qb_done=on_qb_done if last
                                           else None)
                            queue.drain()



    nc.compile()
    return nc


def _prep_core_inputs(x, w_qkv, w_out, b, hg, nhc):
    """Per-core DRAM tensors for batch b, head-group hg."""
    Cc = x.shape[2]
    heads = [hg * nhc + i for i in range(nhc)]
    # wqk columns: per pair: [q_h0|q_h1] tile then [k_h0|k_h1] tile
    qk_rows = []
    for pair in range(nhc // 2):
        for qk in range(2):  # 0 = q, 1 = k
            for hh in range(2):
                hd = heads[pair * 2 + hh]
                qk_rows.append(w_qkv[qk * Cc + hd * 64:qk * Cc + (hd + 1) * 64, :])
    wqk_g = np.ascontiguousarray(np.concatenate(qk_rows, axis=0).T)
    v_rows = [w_qkv[2 * Cc + hd * 64:2 * Cc + (hd + 1) * 64, :] for hd in heads]
    wv_g = np.ascontiguousarray(np.concatenate(v_rows, axis=0).T)
    # wo rows ordered to match attn2 pair layout: pair p = heads (2p, 2p+1)
    wo_rows = [w_out[:, hd * 64:(hd + 1) * 64].T for hd in heads]
    wo_g = np.ascontiguousarray(np.concatenate(wo_rows, axis=0))
    return {
        "wqk": wqk_g.astype(np.float16),
        "wv": wv_g.astype(np.float16),
        "wo": wo_g.astype(np.float16),
    }


def _ensure_ntff_hook():
    """This image's antenv lacks axon_hooks; synthesize the module and
    register the ctypes NTFF profiling hook from trn_agent_boot so
    run_bass_kernel_spmd(trace=True) can capture HW exec time."""
    import types
    try:
        import antenv.axon_hooks  # noqa: F401
        return
    except ImportError:
        pass
    import antenv
    mod = types.ModuleType('antenv.axon_hooks')
    _h = {"hook": None}
    mod.set_axon_ntff_profile_hook = lambda h: _h.__setitem__("hook", h)
    mod.get_axon_ntff_profile_hook = lambda: _h["hook"]
    sys.modules['antenv.axon_hooks'] = mod
    antenv.axon_hooks = mod
    try:
        from trn_agent_boot.trn_boot import _ntff_profile_via_ctypes
        hook = _ntff_profile_via_ctypes('/opt/axon/libaxon_pjrt.so')
        if hook is not None:
            mod.set_axon_ntff_profile_hook(hook)
    except Exception:
        pass


def kernel(x, w_qkv, w_out):
    x = np.asarray(x, dtype=np.float32)
    w_qkv = np.asarray(w_qkv, dtype=np.float32)
    w_out = np.asarray(w_out, dtype=np.float32)

    key = "nc"
    if key not in _cache:
        _cache[key] = _build(T, C, HEADS_PER_CORE, D, N_CORES)
    nc = _cache[key]

    from concourse.bass_utils import run_bass_kernel_spmd

    if TRACE:
        _ensure_ntff_hook()

    tri = np.triu(np.ones((128, 128), dtype=np.float16))
    xTs = [np.ascontiguousarray(x[b].T.astype(np.float16)) for b in range(B)]
    in_maps = []
    for core in range(N_CORES):
        b, hg = core // 4, core % 4
        m = _prep_core_inputs(x, w_qkv, w_out, b, hg, HEADS_PER_CORE)
        m["xT"] = xTs[b]
        m["tri"] = tri
        in_maps.append(m)

    res = run_bass_kernel_spmd(nc, in_maps, core_ids=list(range(N_CORES)),
                               trace=TRACE)
    _cache["last_res"] = res
    partials = [res.results[i]["out"] for i in range(N_CORES)]
    out = np.empty((B, T, C), dtype=np.float32)
    for b in range(B):
        out[b] = np.sum([p.sum(axis=0) for p in partials[4 * b:4 * b + 4]],
                        axis=0)
    return out


# revision 20
# speedup vs baseline: 1.0521x; 1.0521x over previous
"""Causal self-attention (B=2, T=2048, C=1024, H=16) on 8 trn2 NeuronCores.

Sharding: data-parallel on batch (2 groups of 4 cores) x tensor-parallel on
heads (4 heads per core). Each core computes qkv projection for its heads,
causal attention, and a partial out-projection over its heads' slice of the
hidden dim; the host sums the 4 partials per batch.

Device layout notes:
  - x is pre-transposed on host to xT [C, T] so the C contraction sits on
    SBUF partitions for both qkv matmuls.
  - q and k are produced transposed ([64, T] per head) packed in head-pairs
    into [128, T] tiles; v is produced natural [T, 64] with an appended
    ones column so the softmax denominator falls out of the AV matmul
    (row 64 of the PSUM accumulator).
  - scores are computed transposed [T_k, T_q]; softmax runs without max
    subtraction (logits are O(6) for N(0,1) inputs, safe in fp32 exp).
  - matmuls run in float16 (1 cycle/row on the PE like bf16, but 10
    mantissa bits; all intermediate values here are O(3000) max, far from
    the fp16 range limit). PSUM accumulation is fp32.
"""

import sys

for _p in ("/root/.axon_site", "/root/.axon_site/_ro/trn_rl_repo", "/opt/trn_rl_repo"):
    if _p not in sys.path:
        sys.path.append(_p)

import numpy as np

B, T, C = 2, 2048, 1024
H, D = 16, 64
N_CORES = 8
HEADS_PER_CORE = H // 4  # 4 head-groups x 2 batches = 8 cores

_cache = {}
TRACE = False


def _build(T, C, nhc, D, n_cores):
    """Build + compile the SPMD program. nhc = heads per core (must be even)."""
    import concourse.tile as tile
    from concourse import bacc, mybir

    f32 = mybir.dt.float32
    f16 = mybir.dt.float16
    EXP = mybir.ActivationFunctionType.Exp

    assert nhc % 2 == 0 and D == 64
    npairs = nhc // 2
    F = 2 * nhc * D          # qk projection output rows (q+k for nhc heads)
    VW = nhc * D             # v projection width
    CIN = nhc * D            # local c_in slice for out-proj
    NC_ = C // 128           # contraction tiles
    NF = F // 128            # qk f-tiles (= 2*npairs)
    NTB = T // 512           # 512-wide t-blocks
    NTT = T // 128           # 128-wide t-tiles
    NQ = T // 512            # q-blocks
    CO = min(512, C)         # out-proj column block width
    NCO = C // CO            # out-proj column blocks

    nc = bacc.Bacc("TRN2", target_bir_lowering=False, debug=False,
                   enable_asserts=False, num_devices=n_cores)

    xT = nc.dram_tensor("xT", [C, T], f16, kind="ExternalInput").ap()
    wqk = nc.dram_tensor("wqk", [C, F], f16, kind="ExternalInput").ap()
    wv = nc.dram_tensor("wv", [C, VW], f16, kind="ExternalInput").ap()
    wo = nc.dram_tensor("wo", [CIN, C], f16, kind="ExternalInput").ap()
    tri = nc.dram_tensor("tri", [128, 128], f16, kind="ExternalInput").ap()
    out = nc.dram_tensor("out", [npairs, T, C], f32, kind="ExternalOutput").ap()

    with tile.TileContext(nc) as tc:
        with tc.tile_pool(name="qk", bufs=NF) as qk_pool, \
             tc.tile_pool(name="v", bufs=NTT) as v_pool, \
             tc.tile_pool(name="misc", bufs=1) as misc_pool, \
             tc.tile_pool(name="attn2", bufs=npairs) as attn2_pool, \
             tc.tile_pool(name="wo", bufs=npairs) as wo_pool, \
             tc.tile_pool(name="ob", bufs=3) as ob_pool:

            tri_sb = misc_pool.tile([128, 128], f16, tag="tri")
            nc.sync.dma_start(tri_sb[:], tri[:])

            qk_sb = [qk_pool.tile([128, T], f16, tag="qk", name=f"qk{i}") for i in range(NF)]
            v_sb = [v_pool.tile([128, nhc * 65], f16, tag="v", name=f"v{i}") for i in range(NTT)]
            attn2_sb = [attn2_pool.tile([128, T], f16, tag="attn2",
                                        name=f"attn2_{i}") for i in range(npairs)]
            wo_sb = []
            for pr in range(npairs):
                wt = wo_pool.tile([128, C], f16, tag="wo", name=f"wo{pr}")
                nc.sync.dma_start(wt[:], wo[pr * 128:(pr + 1) * 128, :])
                wo_sb.append(wt)

            QW = min(512, T)
            SJT = QW // 128
            NQB = T // QW
            scale = 1.0 / np.sqrt(D)

            def emit_qk_ftile(f, psp, xT_sb, wqk_sb, nbufs=2):
                """Generator: one qk-proj matmul per next() for f-tile `f`,
                WQ-wide t-blocks (fp16 moving operand allows N=1024)."""
                WQ = min(512, T)
                for tb in range(T // WQ):
                    p = psp.tile([128, WQ], f32, tag="psqk", bufs=nbufs,
                                 name=f"psqk_{f}_{tb}")
                    for c in range(NC_):
                        nc.tensor.matmul(
                            p[:],
                            wqk_sb[c][:, f * 128:(f + 1) * 128],
                            xT_sb[c][:, tb * WQ:(tb + 1) * WQ],
                            start=(c == 0), stop=(c == NC_ - 1))
                        yield
                    nc.vector.tensor_copy(
                        qk_sb[f][:, tb * WQ:(tb + 1) * WQ], p[:])

            def emit_outproj_group(pr, tt, psp):
                """Generator: one out-proj matmul per next() for (pr, tt)."""
                ob = ob_pool.tile([128, C], f32, tag="ob", name=f"ob_{pr}_{tt}")
                WO = min(512, C)
                for co in range(C // WO):
                    p = psp.tile([128, WO], f32, tag="psout", bufs=2,
                                 name=f"pso_{pr}_{tt}_{co}")
                    nc.tensor.matmul(
                        p[:],
                        attn2_sb[pr][:, tt * 128:(tt + 1) * 128],
                        wo_sb[pr][:, co * WO:(co + 1) * WO],
                        start=True, stop=True)
                    nc.vector.tensor_copy(ob[:, co * WO:(co + 1) * WO], p[:])
                    yield
                nc.sync.dma_start(out[pr, tt * 128:(tt + 1) * 128, :], ob[:])

            def chain(gens):
                for g in gens:
                    yield from g

            class FillQueue:
                """FIFO of PE-work generators, pulled between attention
                k-tile steps to keep the PE dense while ACT runs exp."""

                def __init__(self):
                    self.gens = []
                    self.nslot = 0
                    self.npulled = 0

                def add(self, gen):
                    self.gens.append(gen)

                def pull(self, spf):
                    self.nslot += 1
                    while self.gens and self.npulled < self.nslot * spf:
                        try:
                            next(self.gens[0])
                            self.npulled += 1
                        except StopIteration:
                            self.gens.pop(0)

                def drain(self):
                    for g in self.gens:
                        for _ in g:
                            pass
                    self.gens = []

            def attention_pair(pair, psS, psV, r_pool, rb_pool, exp_pool,
                               queue, spf, on_qb_done=None):
                """Attention for both heads of `pair`; q-block outer so
                finished q-blocks feed new filler work via on_qb_done.
                Scores for k-tile jt+1 are emitted before the AV matmul of
                k-tile jt; queue fillers cover the residual ACT latency."""
                Q2 = qk_sb[2 * pair]
                K2 = qk_sb[2 * pair + 1]
                for qb in range(NQB):
                    njt = SJT * (qb + 1)
                    for hh in range(2):
                        h = pair * 2 + hh
                        base = hh * 64
                        qT = Q2[base:base + 64, :]
                        kT = K2[base:base + 64, :]
                        av = psV.tile([65, QW], f32, tag="psav",
                                      name=f"psav_{pair}_{hh}_{qb}")

                        def emit_av(jt, ex, col0, ncols, av=av, h=h, njt=njt):
                            nc.tensor.matmul(
                                av[:, col0:col0 + ncols],
                                v_sb[jt][:, h * 65:(h + 1) * 65],
                                ex[:, col0:col0 + ncols],
                                start=(jt == 0), stop=(jt == njt - 1))

                        pend = None
                        for jt in range(njt):
                            d = jt - SJT * qb
                            col0 = max(d, 0) * 128
                            ncols = QW - col0
                            sc = psS.tile([128, QW], f32, tag="pssc",
                                          name=f"pssc_{pair}_{hh}_{qb}_{jt}")
                            nc.tensor.matmul(
                                sc[:, col0:QW],
                                kT[:, jt * 128:(jt + 1) * 128],
                                qT[:, qb * QW + col0:(qb + 1) * QW],
                                start=True, stop=True)
                            ex = exp_pool.tile([128, QW], f16, tag="ex",
                                               bufs=4,
                                               name=f"ex_{pair}_{hh}_{qb}_{jt}")
                            nc.scalar.activation(
                                ex[:, col0:QW], sc[:, col0:QW], EXP,
                                scale=scale)
                            if d >= 0:
                                nc.vector.tensor_mul(
                                    ex[:, col0:col0 + 128],
                                    ex[:, col0:col0 + 128], tri_sb[:])
                            queue.pull(spf)
                            if pend is not None:
                                emit_av(*pend)
                            pend = (jt, ex, col0, ncols)
                        emit_av(*pend)
                        # stage accumulator to SBUF (frees the PSUM bank),
                        # then normalize rows 0..63 by row 64 (sum of exp)
                        avsb = r_pool.tile([65, QW], f32, tag="avs")
                        nc.vector.tensor_copy(avsb[:], av[:])
                        r1s = r_pool.tile([1, QW], f32, tag="r1s")
                        nc.vector.tensor_copy(r1s[:], avsb[64:65, :])
                        r1 = r_pool.tile([1, QW], f32, tag="r1")
                        nc.vector.reciprocal_approx_fast(r1[:], r1s[:])
                        rb = rb_pool.tile([64, QW], f32, tag="rb")
                        nc.gpsimd.partition_broadcast(rb[:], r1[:])
                        nc.vector.tensor_mul(
                            attn2_sb[pair][base:base + 64,
                                           qb * QW:(qb + 1) * QW],
                            avsb[0:64, :], rb[:])
                    if on_qb_done is not None:
                        on_qb_done(qb)

            # ---- phase 1: loads + qk proj (pair 0) + v proj (all) ----
            with tc.tile_pool(name="xT", bufs=NC_) as xT_pool, \
                 tc.tile_pool(name="wqk", bufs=NC_) as wqk_pool, \
                 tc.tile_pool(name="wv", bufs=NC_) as wv_pool:

                xT_sb, wqk_sb, wv_sb = [], [], []
                for c in range(NC_):
                    w1 = wqk_pool.tile([128, F], f16, tag="wqk")
                    nc.sync.dma_start(w1[:], wqk[c * 128:(c + 1) * 128, :])
                    wqk_sb.append(w1)
                    x1 = xT_pool.tile([128, T], f16, tag="xT")
                    nc.sync.dma_start(x1[:], xT[c * 128:(c + 1) * 128, :])
                    xT_sb.append(x1)
                    w2 = wv_pool.tile([128, VW], f16, tag="wv")
                    nc.sync.dma_start(w2[:], wv[c * 128:(c + 1) * 128, :])
                    wv_sb.append(w2)

                with tc.tile_pool(name="psA", bufs=2, space="PSUM") as psA, \
                     tc.tile_pool(name="psB", bufs=2, space="PSUM") as psB:
                    for f in range(min(2, NF)):
                        for _ in emit_qk_ftile(f, psA, xT_sb, wqk_sb):
                            pass
                    for tt in range(NTT):
                        p = psB.tile([128, VW], f32, tag="psv")
                        for c in range(NC_):
                            nc.tensor.matmul(
                                p[:],
                                xT_sb[c][:, tt * 128:(tt + 1) * 128],
                                wv_sb[c][:],
                                start=(c == 0), stop=(c == NC_ - 1))
                        dst = v_sb[tt][:].rearrange("p (h e) -> p h e", e=65)
                        srcp = p[:].rearrange("p (h e) -> p h e", e=64)
                        nc.vector.tensor_copy(dst[:, :, 0:64], srcp)
                        nc.vector.memset(dst[:, :, 64:65], 1.0)

                # ---- phase 2: attention per pair with PE fillers ----
                with tc.tile_pool(name="exp", bufs=4) as exp_pool, \
                     tc.tile_pool(name="rr", bufs=2) as r_pool, \
                     tc.tile_pool(name="rb", bufs=2) as rb_pool, \
                     tc.tile_pool(name="psS", bufs=4, space="PSUM") as psS, \
                     tc.tile_pool(name="psV", bufs=2, space="PSUM") as psV:

                    for pair in range(npairs):
                        with tc.tile_pool(name=f"psF{pair}", bufs=1,
                                          space="PSUM") as psF:
                            queue = FillQueue()
                            last = pair == npairs - 1
                            if not last:
                                # next pair's qk projection
                                queue.add(emit_qk_ftile(
                                    2 * pair + 2, psF, xT_sb, wqk_sb, 1))
                                queue.add(emit_qk_ftile(
                                    2 * pair + 3, psF, xT_sb, wqk_sb, 1))
                            else:
                                # out-proj of all earlier pairs
                                for pr in range(pair):
                                    for tt in range(NTT):
                                        queue.add(emit_outproj_group(
                                            pr, tt, psF))

                            def on_qb_done(qb, pair=pair, psF=psF,
                                           queue=queue):
                                # the last pair's own out-proj becomes
                                # filler as its q-blocks finish
                                ttp = QW // 128
                                for tt in range(qb * ttp, (qb + 1) * ttp):
                                    queue.add(emit_outproj_group(
                                        pair, tt, psF))

                            spf = 0.85 if last else 0.45
                            attention_pair(pair, psS, psV, r_pool, rb_pool,
                                           exp_pool, queue, spf,
                                           on_qb_done=on_qb_done if last
                                           else None)
                            queue.drain()



    nc.compile()
    return nc


def _prep_core_inputs(x, w_qkv, w_out, b, hg, nhc):
    """Per-core DRAM tensors for batch b, head-group hg."""
    Cc = x.shape[2]
    heads = [hg * nhc + i for i in range(nhc)]
    # wqk columns: per pair: [q_h0|q_h1] tile then [k_h0|k_h1] tile
    qk_rows = []
    for pair in range(nhc // 2):
        for qk in range(2):  # 0 = q, 1 = k
            for hh in range(2):
                hd = heads[pair * 2 + hh]
                qk_rows.append(w_qkv[qk * Cc + hd * 64:qk * Cc + (hd + 1) * 64, :])
    wqk_g = np.ascontiguousarray(np.concatenate(qk_rows, axis=0).T)
    v_rows = [w_qkv[2 * Cc + hd * 64:2 * Cc + (hd + 1) * 64, :] for hd in heads]
    wv_g = np.ascontiguousarray(np.concatenate(v_rows, axis=0).T)
    # wo rows ordered to match attn2 pair layout: pair p = heads (2p, 2p+1)
    wo_rows = [w_out[:, hd * 64:(hd + 1) * 64].T for hd in heads]
    wo_g = np.ascontiguousarray(np.concatenate(wo_rows, axis=0))
    return {
        "wqk": wqk_g.astype(np.float16),
        "wv": wv_g.astype(np.float16),
        "wo": wo_g.astype(np.float16),
    }


def _ensure_ntff_hook():
    """This image's antenv lacks axon_hooks; synthesize the module and
    register the ctypes NTFF profiling hook from trn_agent_boot so
    run_bass_kernel_spmd(trace=True) can capture HW exec time."""
    import types
    try:
        import antenv.axon_hooks  # noqa: F401
        return
    except ImportError:
        pass
    import antenv
    mod = types.ModuleType('antenv.axon_hooks')
    _h = {"hook": None}
    mod.set_axon_ntff_profile_hook = lambda h: _h.__setitem__("hook", h)
    mod.get_axon_ntff_profile_hook = lambda: _h["hook"]
    sys.modules['antenv.axon_hooks'] = mod
    antenv.axon_hooks = mod
    try:
        from trn_agent_boot.trn_boot import _ntff_profile_via_ctypes
        hook = _ntff_profile_via_ctypes('/opt/axon/libaxon_pjrt.so')
        if hook is not None:
            mod.set_axon_ntff_profile_hook(hook)
    except Exception:
        pass


def kernel(x, w_qkv, w_out):
    x = np.asarray(x, dtype=np.float32)
    w_qkv = np.asarray(w_qkv, dtype=np.float32)
    w_out = np.asarray(w_out, dtype=np.float32)

    key = "nc"
    if key not in _cache:
        _cache[key] = _build(T, C, HEADS_PER_CORE, D, N_CORES)
    nc = _cache[key]

    from concourse.bass_utils import run_bass_kernel_spmd

    if TRACE:
        _ensure_ntff_hook()

    tri = np.triu(np.ones((128, 128), dtype=np.float16))
    xTs = [np.ascontiguousarray(x[b].T.astype(np.float16)) for b in range(B)]
    in_maps = []
    for core in range(N_CORES):
        b, hg = core // 4, core % 4
        m = _prep_core_inputs(x, w_qkv, w_out, b, hg, HEADS_PER_CORE)
        m["xT"] = xTs[b]
        m["tri"] = tri
        in_maps.append(m)

    res = run_bass_kernel_spmd(nc, in_maps, core_ids=list(range(N_CORES)),
                               trace=TRACE)
    _cache["last_res"] = res
    partials = [res.results[i]["out"] for i in range(N_CORES)]
    out = np.empty((B, T, C), dtype=np.float32)
    for b in range(B):
        out[b] = np.sum([p.sum(axis=0) for p in partials[4 * b:4 * b + 4]],
                        axis=0)
    return out
